# revision 36
# baseline (speedup 1.0000x reference)
"""Grok1-style attention on 8 trn2 NeuronCores, tensor-parallel over heads.

Sharding (per core c of 8): q heads 4c..4c+3, kv head c. w_qkv column-sharded
(768 features/core), w_o row-sharded; partial o_proj outputs summed on host.

v2 design notes (vs v1 baseline at ~560us):
  - Single interleaved PE emission stream: attention for q-tile qt and
    o_proj for qt-1 are pumped between the qkv matmul chunks of block
    tt=qt+1, so Scalar's tanh+exp (~210us) hides under the PE stream and
    the PE DVFS ramp (0.65->2.4GHz after 3us continuous) stays hot.
  - Causal mask via gpsimd.affine_select on the (otherwise idle) Pool
    engine (fill 0 post-exp) instead of vector multiplies.
  - Softmax denominator: ones-vector matmul accumulated in psum; 1/d via
    vector.reciprocal_approx_fast (psum read, ~3e-6 rel), broadcast via
    gpsimd.partition_broadcast, one vector mult psum*sbuf -> normalized.
  - Inputs pre-cast to bf16 on host (halves HBM read); outputs written
    fp16 (halves write; partials summed on host in fp32).
  - All resident loads chunked so the first qkv matmul starts ~2us in.
  - fp8 was measured (DoubleRow = 2x bf16 FLOPs on real HW) but every
    injection point alone exceeds the 2e-2 rel-err budget: wo 3.9%,
    probs 2.7%, v 2.7%, qk 4.7%. bf16 everywhere on the PE.
"""
import numpy as np
import ml_dtypes
from contextlib import ExitStack

import concourse.bass as bass
import concourse.mybir as mybir
import concourse.tile as tile
from concourse import bacc
from concourse.bass_utils import run_bass_kernel_spmd
from concourse.masks import make_identity

T = 2048
D = 4096
HD = 128
HALF = 64
NCORES = 8
HPC = 4                    # q heads per core
QF = HPC * HD              # 512
NF = QF + 2 * HD           # 768 qkv features per core
NCH = D // 128             # 32 contraction chunks
TT = 512                   # t-tile width (matmul moving dim)
NTT = T // TT              # 4
NKT = T // 128             # 16 k-tiles
SCALING = HD ** -0.5
CAP = 30.0
BF = mybir.dt.bfloat16
F32 = mybir.dt.float32
F16 = mybir.dt.float16
AF = mybir.ActivationFunctionType
ALU = mybir.AluOpType


def _emit(nc):
    # host pre-arranges inputs partition-major so every DMA slice is one
    # large contiguous run per partition (few descriptors, full bandwidth)
    hX = nc.dram_tensor("hX", [128, NTT, NCH, TT], BF, kind="ExternalInput").ap()
    wqX = nc.dram_tensor("wqX", [128, NCH, NF], BF, kind="ExternalInput").ap()
    woX = nc.dram_tensor("woX", [128, HPC, D], BF, kind="ExternalInput").ap()
    cc = nc.dram_tensor("cc", [HD, T], BF, kind="ExternalInput").ap()
    ss = nc.dram_tensor("ss", [HD, T], BF, kind="ExternalInput").ap()
    out = nc.dram_tensor("out", [T, D], F16, kind="ExternalOutput").ap()

    with tile.TileContext(nc) as tc:
        with ExitStack() as ctx:
            wqp = ctx.enter_context(tc.tile_pool(name="wqp", bufs=1))
            wop = ctx.enter_context(tc.tile_pool(name="wop", bufs=1))
            cstp = ctx.enter_context(tc.tile_pool(name="cstp", bufs=1))
            hqp = ctx.enter_context(tc.tile_pool(name="hqp", bufs=4))
            seqp = ctx.enter_context(tc.tile_pool(name="seqp", bufs=1))
            vtp = ctx.enter_context(tc.tile_pool(name="vtp", bufs=2))
            rtp = ctx.enter_context(tc.tile_pool(name="rtp", bufs=3))
            etp = ctx.enter_context(tc.tile_pool(name="etp", bufs=3))
            smp = ctx.enter_context(tc.tile_pool(name="smp", bufs=2))
            obp = ctx.enter_context(tc.tile_pool(name="obp", bufs=3))
            psp = ctx.enter_context(tc.tile_pool(name="psp", bufs=1, space="PSUM"))

            # ---- constants + resident loads (chunked) ----
            cc_sb = cstp.tile([HD, T], BF, tag="cc")
            ss_sb = cstp.tile([HD, T], BF, tag="ss")
            ident = cstp.tile([128, 128], BF, tag="id")
            make_identity(nc, ident[:])
            ones_k = cstp.tile([128, 1], BF, tag="ones_k")
            nc.vector.memset(ones_k[:], 1.0)

            # h quarters: [128, 8, 512] per (tt, qtr)
            hq = {}

            def prefetch_h(tt, qtr, eng, split=1):
                t_ = hqp.tile([128, 8, TT], BF, tag="hq", name=f"h{tt}_{qtr}")
                step = 8 // split
                for s in range(split):
                    eng.dma_start(
                        t_[:, s * step:(s + 1) * step, :],
                        hX[:, tt, 8 * qtr + s * step:8 * qtr + (s + 1) * step,
                           :])
                hq[(tt, qtr)] = t_

            # startup: the first-needed pieces lead their queues (per-core
            # HBM is ~358GB/s shared, so arrival order ~= issue order)
            wq_sb = wqp.tile([128, NCH, NF], BF, tag="wq")
            wo_sb = wop.tile([128, HPC, D], BF, tag="wo")
            nc.gpsimd.dma_start(wq_sb[:, 0:2, :], wqX[:, 0:2, :])
            prefetch_h(0, 0, nc.sync, split=4)
            nc.gpsimd.dma_start(wq_sb[:, 2:4, :], wqX[:, 2:4, :])
            prefetch_h(0, 1, nc.sync)
            for j in range(1, 4):  # chunks c=4..15 on gpsimd
                nc.gpsimd.dma_start(
                    wq_sb[:, 4 * j:4 * j + 4, :], wqX[:, 4 * j:4 * j + 4, :])
            prefetch_h(0, 2, nc.scalar)
            prefetch_h(0, 3, nc.scalar)
            for j in range(4, 8):  # chunks c=16..31 on scalar queue
                nc.scalar.dma_start(
                    wq_sb[:, 4 * j:4 * j + 4, :], wqX[:, 4 * j:4 * j + 4, :])
            nc.sync.dma_start(cc_sb[:], cc[:, :])
            nc.sync.dma_start(ss_sb[:], ss[:, :])
            nc.gpsimd.dma_start(wo_sb[:, 0:2, :], woX[:, 0:2, :])
            nc.scalar.dma_start(wo_sb[:, 2:4, :], woX[:, 2:4, :])

            # persistent per-sequence tiles
            qTt = [[seqp.tile([HD, TT], BF, tag=f"q{h}_{t}", name=f"qT{h}_{t}")
                    for t in range(NTT)] for h in range(HPC)]
            kTt = [seqp.tile([HD, TT], BF, tag=f"k_{t}", name=f"kT{t}")
                   for t in range(NTT)]
            vbt = [seqp.tile([128, HD], BF, tag=f"vb_{kt}", name=f"vb{kt}")
                   for kt in range(NKT)]
            atq = [[seqp.tile([HD, TT], BF, tag=f"a{h}_{t}", name=f"at{h}_{t}")
                    for t in range(NTT)] for h in range(HPC)]

            # ---------- unit machinery ----------
            # Each unit is a closure that emits instructions and may return a
            # deferred second stage. The pump runs the deferred stage of unit
            # N one slot after unit N's first stage, so the PE's in-order
            # stream never blocks on Scalar's tanh->exp chain: the score
            # matmul of unit N+1 and a few qkv chunks sit between exp(N) and
            # the accumulating matmuls that consume it.
            units = []
            state = {"sidx": 0, "dma_flip": 0, "pending": None, "debt": 0.0}
            actx = {}  # (h, qt) -> (a_ps, d_ps)

            def pump(n):
                for _ in range(min(n, len(units))):
                    u = units.pop(0)
                    if state["pending"] is not None:
                        state["pending"]()
                        state["pending"] = None
                    state["pending"] = u()

            def flush():
                pump(len(units))
                if state["pending"] is not None:
                    state["pending"]()
                    state["pending"] = None

            def attn_unit(h, qt, kt, sbanks=("b4", "b5")):
                def emit():
                    nkt = 4 * qt + 4
                    m = kt - 4 * qt
                    j0 = 128 * m if m >= 0 else 0
                    nw = TT - j0
                    if kt == 0:
                        actx[(h, qt)] = (
                            psp.tile([HD, TT], F32, tag="b6",
                                     name=f"aps{h}_{qt}"),
                            psp.tile([1, TT], F32, tag="b7",
                                     name=f"dps{h}_{qt}"),
                        )
                    a_ps, d_ps = actx[(h, qt)]
                    sb = sbanks[state["sidx"] % len(sbanks)]
                    state["sidx"] += 1
                    s_ps = psp.tile([128, TT], F32, tag=sb, name="s_ps")
                    nc.tensor.matmul(
                        s_ps[:, j0:TT],
                        kTt[kt // 4][:, (kt % 4) * 128:(kt % 4 + 1) * 128],
                        qTt[h][qt][:, j0:TT], start=True, stop=True)
                    nc.scalar.activation(
                        s_ps[:, j0:TT], s_ps[:, j0:TT], AF.Tanh,
                        scale=SCALING / CAP)
                    et = etp.tile([128, TT], BF, tag="et")
                    nc.scalar.activation(
                        et[:, j0:TT], s_ps[:, j0:TT], AF.Exp, scale=CAP)
                    if m >= 0:
                        # causal: keep where q_global - k_global = j - p >= 0
                        nc.gpsimd.affine_select(
                            et[:, j0:TT], et[:, j0:TT], pattern=[[1, nw]],
                            compare_op=ALU.is_ge, fill=0.0, base=0,
                            channel_multiplier=-1)

                    def acc():
                        last = kt == nkt - 1
                        nc.tensor.matmul(a_ps[:, j0:TT], vbt[kt][:],
                                         et[:, j0:TT],
                                         start=(kt == 0), stop=last)
                        nc.tensor.matmul(d_ps[:, j0:TT], ones_k[:],
                                         et[:, j0:TT],
                                         start=(kt == 0), stop=last)
                    return acc
                return emit

            def norm_unit(h, qt):
                def emit():
                    a_ps, d_ps = actx.pop((h, qt))
                    rec = smp.tile([1, TT], F32, tag="rec")
                    nc.vector.reciprocal_approx_fast(rec[:], d_ps[:])
                    pbc = smp.tile([128, TT], F32, tag="pbc")
                    nc.gpsimd.partition_broadcast(pbc[:], rec[:])
                    nc.vector.tensor_tensor(atq[h][qt][:], a_ps[:], pbc[:],
                                            ALU.mult)
                return emit

            def oproj_unit(qt, i, g, bank="b3", vec_only=False):
                def emit():
                    t0 = (4 * qt + i) * 128
                    n0 = g * TT
                    pls = psp.tile([128, TT], F32, tag=bank,
                                   name=f"o{qt}_{i}_{g}")
                    for fc in range(HPC):
                        nc.tensor.matmul(
                            pls[:], atq[fc][qt][:, i * 128:(i + 1) * 128],
                            wo_sb[:, fc, n0:n0 + TT],
                            start=(fc == 0), stop=(fc == HPC - 1))
                    ob = obp.tile([128, TT], F16, tag="ob")
                    k = state["dma_flip"]
                    state["dma_flip"] += 1
                    if vec_only or k % 2 == 0:
                        nc.vector.tensor_copy(ob[:], pls[:])
                    else:
                        nc.scalar.activation(ob[:], pls[:], AF.Copy)

                    def dma():
                        # deferred one pump slot: by then ob is written, so
                        # this never head-of-line-blocks its DMA queue
                        eng = (nc.sync, nc.gpsimd)[k % 2]
                        eng.dma_start(out[t0:t0 + 128, n0:n0 + TT], ob[:])
                    return dma
                return emit

            def queue_attn(qt, heads, kts, sbanks=("b4", "b5"), to=None):
                dst = units if to is None else to
                kts = list(kts)
                for h in heads:
                    for kt in kts:
                        dst.append(attn_unit(h, qt, kt, sbanks))
                    if kts[-1] == 4 * qt + 3:
                        dst.append(norm_unit(h, qt))

            def queue_oproj(qt, banks=("b3",), to=None, vec_only=False):
                dst = units if to is None else to
                for i in range(4):
                    for g in range(8):
                        dst.append(
                            oproj_unit(qt, i, g, banks[(8 * i + g) % len(banks)],
                                       vec_only))

            # ---------- rope / v processing after a qkv trio ----------
            def rope_trio(fg, tt, ps3):
                # Phase 1: one psum read per feature, split across V and S,
                # emitted before everything else so the trio's psum banks
                # free after ~2 engine ops (the next trio's start gates on
                # them). Phase 2: the rope math runs from the sbuf copies.
                t0 = tt * TT
                cps = []
                for j in range(3):
                    f = fg * 3 + j
                    tag, pool = ("vt", vtp) if f == 5 else ("qks", rtp)
                    cp = pool.tile([128, TT], BF, tag=tag)
                    if j == 1:
                        nc.scalar.activation(cp[:], ps3[j][:], AF.Copy)
                    else:
                        nc.vector.tensor_copy(cp[:], ps3[j][:])
                    cps.append(cp)
                for j in range(3):
                    f = fg * 3 + j
                    qks = cps[j]
                    if f < 5:
                        rot = rtp.tile([128, TT], BF, tag="rot")
                        nc.sync.dma_start(rot[0:HALF, :], qks[HALF:128, :])
                        nc.sync.dma_start(rot[HALF:128, :], qks[0:HALF, :])
                        m1 = rtp.tile([128, TT], BF, tag="m1")
                        nc.vector.tensor_tensor(m1[:], qks[:],
                                                cc_sb[:, t0:t0 + TT], ALU.mult)
                        m2 = rtp.tile([128, TT], BF, tag="m2")
                        nc.gpsimd.tensor_tensor(m2[:], rot[:],
                                                ss_sb[:, t0:t0 + TT], ALU.mult)
                        dst = qTt[f][tt] if f < HPC else kTt[tt]
                        nc.vector.tensor_tensor(dst[:], m1[:], m2[:], ALU.add)
                    else:
                        for i in range(4):
                            tp = psp.tile([128, 128], BF, tag="b3", name="tp")
                            nc.tensor.transpose(
                                tp[:], qks[:, i * 128:(i + 1) * 128], ident[:])
                            nc.vector.tensor_copy(vbt[4 * tt + i][:], tp[:])

            def queue_mixed(alist, olist):
                # interleave S-heavy attention units with S-free o_proj
                # units so Scalar is never handed a long consecutive burst
                na, no = len(alist), len(olist)
                ia = io = 0
                while ia < na or io < no:
                    # keep the emitted prefix at the global ratio
                    if ia * no <= io * na and ia < na:
                        units.append(alist[ia]); ia += 1
                    elif io < no:
                        units.append(olist[io]); io += 1
                    else:
                        units.append(alist[ia]); ia += 1

            # ---------- main blocks ----------
            for tt in range(NTT):
                # queue work that becomes runnable this block
                if tt == 1:
                    queue_attn(0, range(HPC), range(4))
                elif tt == 2:
                    a, o = [], []
                    queue_attn(1, range(HPC), range(8), to=a)
                    queue_oproj(0, to=o)
                    queue_mixed(a, o)
                elif tt == 3:
                    a, o = [], []
                    queue_attn(2, range(HPC), range(12), to=a)
                    queue_oproj(1, to=o)
                    queue_mixed(a, o)
                    # first half of o_proj(2) follows the whole attn(2)
                    # batch (it needs all four norms(2)); the rest stays in
                    # the tail as PE filler between attention units
                    for i in range(2):
                        for g in range(8):
                            units.append(oproj_unit(2, i, g, "b3"))

                for fg in range(2):
                    ps3 = [psp.tile([128, TT], F32, tag=f"b{j}",
                                    name=f"qkv{tt}_{fg}_{j}")
                           for j in range(3)]
                    # stop pumping a few chunks before the trio ends so the
                    # Vector/Scalar queues drain and the rope psum-reads (which
                    # gate the next trio's banks) execute promptly.
                    nslots = 24
                    for c in range(NCH):
                        src = hq[(tt, c // 8)][:, c % 8, :]
                        for j in range(3):
                            f = fg * 3 + j
                            nc.tensor.matmul(
                                ps3[j][:],
                                wq_sb[:, c, f * 128:(f + 1) * 128],
                                src, start=(c == 0), stop=(c == NCH - 1))
                        if fg == 1 and c % 8 == 7 and tt < NTT - 1:
                            # h(tt, c//8) just had its last read emitted; its
                            # slot frees shortly, so this prefetch won't block
                            # the sync queue for long.
                            prefetch_h(tt + 1, c // 8, nc.sync)
                        if c < nslots:
                            # fractional-debt pacing over the remaining slots
                            # of the whole block: units arrive at a steady
                            # rate instead of bursts that put Scalar several
                            # units behind the PE stream
                            rem_slots = (nslots - c) + (nslots if fg == 0 else 0)
                            state["debt"] += len(units) / rem_slots
                            n = int(state["debt"])
                            if n:
                                state["debt"] -= n
                                pump(min(n, 3))
                    rope_trio(fg, tt, ps3)
                    if tt == 3 and fg == 0:
                        # q0 of tile 3 now exists; h0's non-diagonal attention
                        # can overlap fg1 of block 3. (Only one head: the
                        # b6/b7 accumulators are single-buffered, so head h+1
                        # must not start before norm(h) is emitted.)
                        queue_attn(3, [0], range(12))

            # ---------- tail ----------
            # remaining attention (strictly head-sequential for b6/b7),
            # interleaved 1:1 with o_proj(2) groups as PE filler.
            ta = []
            queue_attn(3, [0], range(12, 16), sbanks=("b4", "b5", "b0"), to=ta)
            for h in range(1, HPC):
                queue_attn(3, [h], range(16), sbanks=("b4", "b5", "b0"), to=ta)
            to2 = []
            for i in range(2, 4):
                for g in range(8):
                    to2.append(oproj_unit(2, i, g, ("b3", "b1")[g % 2],
                                          vec_only=True))
            while ta or to2:
                if ta:
                    units.append(ta.pop(0))
                if to2:
                    units.append(to2.pop(0))
            # o_proj(3) needs all four heads' normalized outputs: last.
            queue_oproj(3, banks=("b1", "b2", "b3", "b0"), vec_only=True)
            flush()
    return nc


_CACHE = {}


def _get_nc():
    if "nc" not in _CACHE:
        nc = bacc.Bacc("TRN2", target_bir_lowering=False, debug=False)
        _emit(nc)
        nc.compile()
        _CACHE["nc"] = nc
    return _CACHE["nc"]


def _in_maps(positions, hidden_states, w_qkv, w_o):
    bf16 = ml_dtypes.bfloat16
    hidden_states = np.asarray(hidden_states, dtype=np.float32)
    w_qkv = np.asarray(w_qkv, dtype=np.float32)
    w_o = np.asarray(w_o, dtype=np.float32)
    pos = np.asarray(positions).astype(np.float64)

    # hX[p, tt, ch, tin] = hidden[512*tt + tin, 128*ch + p]
    hX = np.ascontiguousarray(
        hidden_states.reshape(NTT, TT, NCH, 128).transpose(3, 0, 2, 1)
    ).astype(bf16)
    inv_freq = 1.0 / (10000.0 ** (np.arange(HALF, dtype=np.float64) * 2.0 / HD))
    ang = np.outer(inv_freq, pos)                      # [64, T]
    cos = np.cos(ang).astype(np.float32)
    sin = np.sin(ang).astype(np.float32)
    cc = np.ascontiguousarray(
        np.concatenate([cos, cos], axis=0)).astype(bf16)   # [128, T]
    ss = np.ascontiguousarray(
        np.concatenate([-sin, sin], axis=0)).astype(bf16)  # [128, T]

    in_maps = []
    for c in range(NCORES):
        rows = np.concatenate([
            w_qkv[QF * c:QF * (c + 1)],
            w_qkv[D + HD * c:D + HD * (c + 1)],
            w_qkv[D + HD * NCORES + HD * c:D + HD * NCORES + HD * (c + 1)],
        ], axis=0)                                      # [768, 4096]
        # wqX[p, ch, f] = rows[f, 128*ch + p]
        wqX = np.ascontiguousarray(
            rows.reshape(NF, NCH, 128).transpose(2, 1, 0)).astype(bf16)
        wo_c = w_o[:, QF * c:QF * (c + 1)].T            # [512, 4096]
        # woX[p, h, n] = wo_c[128*h + p, n]
        woX = np.ascontiguousarray(
            wo_c.reshape(HPC, 128, D).transpose(1, 0, 2)).astype(bf16)
        in_maps.append({"hX": hX, "wqX": wqX, "woX": woX, "cc": cc, "ss": ss})
    return in_maps


def run(positions, hidden_states, w_qkv, w_o, trace=False):
    nc = _get_nc()
    in_maps = _in_maps(positions, hidden_states, w_qkv, w_o)
    res = run_bass_kernel_spmd(nc, in_maps, list(range(NCORES)), trace=trace)
    parts = np.stack([np.asarray(res.results[i]["out"], dtype=np.float32)
                      for i in range(NCORES)], axis=0)
    full = parts.sum(axis=0).astype(np.float32)
    return full, res


def kernel(positions, hidden_states, w_qkv, w_o):
    full, _ = run(positions, hidden_states, w_qkv, w_o, trace=False)
    return full
